# revision 14
# baseline (speedup 1.0000x reference)
"""Trainium2 Bass kernel for nn_L2Net (Jeffress coincidence-detector SNN).

Contract: kernel(**inputs) takes the FULL unsharded inputs (numpy) and
returns the FULL (T, N, 1) float32 output.

Strategy: pure data parallelism over the batch axis N=32 -> 4 samples on
each of 8 NeuronCores (every state in the model is per-sample, so there
are no collectives). Inside a core:
  partitions = C (=128 channels), free dims = (n_local, d / k / t).
  - Jeffress LIF: 3 fused ops/step; pre-reset membranes streamed to SBUF,
    spikes * kint recovered afterwards with one fused compare-multiply and
    a strided reduce (chunked so the vi chain can start early).
  - IF neurons: 2 fused ops/step (scalar_tensor_tensor integrate + reset).
  - SynapseFilters: one masked tensor_tensor_scan per filter (decay mask
    carries 0.0 at segment starts to reset the recurrence between samples).
  - Linear layers of sqrt_model + the sum over C: PE matmuls.
  - Output q2: cumulative-sum scan, DMAed out once.
"""
import os
import sys

import numpy as np

sys.path.insert(0, "/opt/trn_rl_repo")

T, N, C, D = 64, 32, 128, 64
NCORES = 8
NL = N // NCORES          # samples per core
TAU = np.float32(20.0)    # jeffress LIF tau
F32 = np.float32

_cache = {}


def _build_program():
    import concourse.bass as bass
    import concourse.bacc as bacc
    import concourse.mybir as mybir
    import concourse.tile as tile

    dt = mybir.dt.float32
    op = mybir.AluOpType
    AP = bass.AP

    nc = bacc.Bacc("TRN2", target_bir_lowering=False, debug=False,
                   num_devices=NCORES)

    # ---------------- DRAM I/O ----------------
    x0R_d = nc.dram_tensor("x0r", [C, NL, 128], dt, kind="ExternalInput")
    x1P_d = nc.dram_tensor("x1p", [C, NL, 128], dt, kind="ExternalInput")
    kint_d = nc.dram_tensor("kint", [C, D], dt, kind="ExternalInput")
    # packed per-channel weights: cols 0:10 w1, 10:20 b1, 20:30 w2, 30 b2,
    # col 31 ones (for the C-sum matmul)
    wpk_d = nc.dram_tensor("wpk", [C, 32], dt, kind="ExternalInput")
    # packed sqrt-model weights on 32 partitions:
    # col 0 sw2T, col 1 sb0, col 2 sb1, cols 3:35 sw1T (sw1T[k,j]=sw1[j,k])
    spk_d = nc.dram_tensor("spk", [32, 35], dt, kind="ExternalInput")
    # row tile: cols 0:32 sw0 (as [1,32]), col 32 sb2
    srow_d = nc.dram_tensor("srow", [1, 33], dt, kind="ExternalInput")
    out_d = nc.dram_tensor("out", [NL, T], dt, kind="ExternalOutput")

    NT = NL * T               # 256
    NKT = NL * 10 * T         # 2560
    CH = 8                    # t-chunks for the jeffress bulk
    CHT = T // CH             # 8 steps per chunk

    with tile.TileContext(nc) as tc:
        with (
            tc.tile_pool(name="pool", bufs=1) as pool,
            tc.tile_pool(name="psum", bufs=1, space="PSUM") as psum,
        ):
            xa = pool.tile([C, NL, 128], dt)
            xb = pool.tile([C, NL, 128], dt)
            kint = pool.tile([C, D], dt)
            wpk = pool.tile([C, 32], dt)
            spk = pool.tile([32, 35], dt)
            srow = pool.tile([1, 33], dt)
            mask1 = pool.tile([C, NT], dt)
            mask40 = pool.tile([C, NKT], dt)
            cmask = pool.tile([1, NT], dt)

            for tl, dr in ((xa, x0R_d), (xb, x1P_d), (kint, kint_d),
                           (wpk, wpk_d), (spk, spk_d), (srow, srow_d)):
                nc.sync.dma_start(tl[:], dr[:])

            # filter decay masks built on-device: 0.5 everywhere with 0.0 at
            # each t-segment start (resets the scan between samples);
            # cmask likewise with 1.0 for the output cumsum.
            nc.vector.memset(mask1[:], 0.5)
            nc.vector.memset(
                mask1[:].rearrange("c (n t) -> c n t", t=T)[:, :, 0:1], 0.0)
            nc.gpsimd.memset(mask40[:], 0.5)
            nc.gpsimd.memset(
                mask40[:].rearrange("c (s t) -> c s t", t=T)[:, :, 0:1], 0.0)
            nc.vector.memset(cmask[:], 1.0)
            nc.vector.memset(
                cmask[:].rearrange("p (n t) -> p n t", t=T)[:, :, 0:1], 0.0)

            # jeffress state: GPSIMD builds u_t = a_t + b_t (tensor_tensor
            # is Pool-legal), DVE runs the 2-op fused LIF chain.
            vj = pool.tile([C, NL, D], dt)
            vjs = pool.tile([C, T, NL, D], dt)   # pre-reset membrane stream
            zc = pool.tile([C, T, NL], dt)       # jeffress->vi inputs

            vi = pool.tile([C, NL], dt)
            vis = pool.tile([C, NL, T], dt)      # (n,t) pre-reset stream
            f1s = pool.tile([C, NT], dt)
            tmp1 = pool.tile([C, NKT], dt)       # (n,k,t) v1 inputs
            v1 = pool.tile([C, NL, 10], dt)
            v1s = pool.tile([C, NL, 10, T], dt)  # (n,k,t)
            f2s = pool.tile([C, NKT], dt)
            m2 = pool.tile([C, NKT], dt)         # (n,t,k)
            red2 = pool.tile([C, NL, T], dt)
            v2 = pool.tile([C, NL], dt)
            v2s = pool.tile([C, NL, T], dt)
            fss = pool.tile([C, NT], dt)

            vs = pool.tile([1, NL], dt)
            vss = pool.tile([1, NL, T], dt)
            q0 = pool.tile([32, NL], dt)
            q0s = pool.tile([32, NL, T], dt)
            g1s = pool.tile([32, NT], dt)
            tq0 = pool.tile([32, NT], dt)
            tq1 = pool.tile([32, NT], dt)
            tq2 = pool.tile([1, NT], dt)
            q1 = pool.tile([32, NL], dt)
            q1s = pool.tile([32, NL, T], dt)
            g2s = pool.tile([32, NT], dt)
            q2s = pool.tile([1, NT], dt)

            sums_ps = psum.tile([1, NT], dt)
            q0_ps = psum.tile([32, NT], dt)
            q1_ps = psum.tile([32, NT], dt)
            q2_ps = psum.tile([1, NT], dt)

            for tl in (vj, vi, v1, v2, vs, q0, q1):
                nc.vector.memset(tl[:], 0.0)

            dec = F32(1.0) - F32(1.0) / TAU     # 0.95

            # ---------------- phase 1: jeffress LIF ----------------
            # GPSIMD: u_t = a_t + b_t (delay-line windows; pre-reversed /
            # padded / prescaled by 1/tau on the host).
            # DVE: vn = (vj*0.95) + u_t ; vj = (vn<1)*vn   (2 fused ops)
            # bulk (chunked): GPSIMD turns the membrane stream into
            # kint-weighted spikes in place, DVE reduces over d -> zc.
            for ch in range(CH):
                t0 = ch * CHT
                for t in range(t0, t0 + CHT):
                    u_t = pool.tile([C, NL, D], dt, tag="u", bufs=4)
                    nc.gpsimd.tensor_tensor(
                        u_t[:], xa[:, :, 63 - t:127 - t],
                        xb[:, :, t:t + 64], op.add)
                    vn = vjs[:, t]
                    nc.vector.scalar_tensor_tensor(vn, vj[:], float(dec),
                                                   u_t[:], op.mult, op.add)
                    nc.vector.scalar_tensor_tensor(vj[:], vn, 1.0, vn,
                                                   op.is_lt, op.mult)
                blk = vjs[:, t0:t0 + CHT].rearrange("c a b d -> c (a b) d")
                kb = kint[:].unsqueeze(1).broadcast_to((C, CHT * NL, D))
                nc.gpsimd.tensor_scalar(blk, blk, 1.0, None, op.is_ge)
                nc.gpsimd.tensor_tensor(blk, blk, kb, op.mult)
                nc.vector.tensor_reduce(
                    zc[:, t0:t0 + CHT], blk.rearrange(
                        "c (a b) d -> c a b d", a=CHT),
                    mybir.AxisListType.X, op.add)

            # ---------------- phase 2: vi integrate-and-fire -------------
            for t in range(T):
                vn = vis[:, :, t]
                nc.vector.tensor_tensor(vn, vi[:], zc[:, t], op.add)
                nc.vector.scalar_tensor_tensor(vi[:], vn, 1.0, vn,
                                               op.is_lt, op.mult)
            s2 = vis[:].rearrange("c n t -> c (n t)")
            nc.gpsimd.tensor_scalar(s2, s2, 1.0, None, op.is_ge)

            # f1 filter: one masked scan over (n,t)
            nc.vector.tensor_tensor_scan(f1s[:], mask1[:], s2, 0.0,
                                         op.mult, op.add)

            # v1 inputs: tmp1[c,n,k,t] = f1[c,n,t]*w1[k] + b1[k]
            f1b = f1s[:].rearrange("c (n t) -> c n t", n=NL) \
                .unsqueeze(2).broadcast_to((C, NL, 10, T))
            w1b = wpk[:, 0:10].unsqueeze(1).unsqueeze(3) \
                .broadcast_to((C, NL, 10, T))
            b1b = wpk[:, 10:20].unsqueeze(1).unsqueeze(3) \
                .broadcast_to((C, NL, 10, T))
            t1v = tmp1[:].rearrange("c (n k t) -> c n k t", n=NL, k=10)
            nc.vector.tensor_tensor(t1v, f1b, w1b, op.mult)
            nc.vector.tensor_tensor(t1v, t1v, b1b, op.add)

            # ---------------- phase 3: v1 IF chain -----------------------
            t1r = tmp1[:].rearrange("c (n k t) -> c n k t", n=NL, k=10)
            for t in range(T):
                vn = v1s[:, :, :, t]
                nc.vector.tensor_tensor(vn, v1[:], t1r[:, :, :, t], op.add)
                nc.vector.scalar_tensor_tensor(v1[:], vn, 1.0, vn,
                                               op.is_lt, op.mult)
            s3 = v1s[:].rearrange("c n k t -> c (n k t)")
            nc.vector.tensor_scalar(s3, s3, 1.0, None, op.is_ge)

            # f2 filter: masked scan over all (n,k) segments
            nc.vector.tensor_tensor_scan(f2s[:], mask40[:], s3, 0.0,
                                         op.mult, op.add)

            # v2 inputs: m2[c,n,t,k] = f2[c,n,k,t]*w2[k]; red2 = sum_k + b2
            f2v = f2s[:].rearrange("c (n k t) -> c n k t", n=NL, k=10)
            w2b = wpk[:, 20:30].unsqueeze(1).unsqueeze(3) \
                .broadcast_to((C, NL, 10, T))
            # m2 stored (n,t,k) so the k-reduce is innermost; write it from
            # the (n,k,t) iteration via a transposed view
            m2v = m2[:].rearrange("c (n t k) -> c n t k", n=NL,
                                  t=T).transpose([0, 1, 3, 2])
            nc.vector.tensor_tensor(m2v, f2v, w2b, op.mult)
            nc.vector.tensor_reduce(
                red2[:], m2[:].rearrange("c (nt k) -> c nt k", k=10),
                mybir.AxisListType.X, op.add)

            # ---------------- phase 4: v2 IF chain -----------------------
            b2ap = wpk[:, 30:31]
            for t in range(T):
                vn = v2s[:, :, t]
                nc.vector.scalar_tensor_tensor(vn, v2[:], b2ap,
                                               red2[:, :, t], op.add, op.add)
                nc.vector.scalar_tensor_tensor(v2[:], vn, 1.0, vn,
                                               op.is_lt, op.mult)
            s4 = v2s[:].rearrange("c n t -> c (n t)")
            nc.vector.tensor_scalar(s4, s4, 1.0, None, op.is_ge)

            # fs filter + sum over channels (PE)
            nc.vector.tensor_tensor_scan(fss[:], mask1[:], s4, 0.0,
                                         op.mult, op.add)
            nc.tensor.matmul(sums_ps[:], wpk[:, 31:32], fss[:])
            sums_sb = pool.tile([1, NT], dt)
            nc.vector.tensor_scalar(sums_sb[:], sums_ps[:], 0.0, None,
                                    op.add)   # PSUM->SBUF (gpsimd can't PSUM)

            # ---------------- phase 5: vs IF chain -----------------------
            sums_v = sums_sb[:].rearrange("p (n t) -> p n t", n=NL)
            vsm = pool.tile([1, NL], dt)
            for t in range(T):
                vn = vss[:, :, t]
                nc.gpsimd.tensor_tensor(vn, vs[:], sums_v[:, :, t], op.add)
                nc.gpsimd.tensor_scalar(vsm[:], vn, 1.0, None, op.is_lt)
                nc.gpsimd.tensor_tensor(vs[:], vsm[:], vn, op.mult)
            hsv = vss[:].rearrange("p n t -> p (n t)")
            nc.gpsimd.tensor_scalar(hsv, hsv, 1.0, None, op.is_ge)

            # q0 inputs: sw0 outer h (PE, K=1) + sb0
            nc.tensor.matmul(q0_ps[:], srow[:, 0:32], hsv)
            nc.vector.tensor_scalar(tq0[:], q0_ps[:], spk[:, 1:2], None,
                                    op.add)

            # ---------------- phase 6: q0 IF chain -----------------------
            tq0v = tq0[:].rearrange("p (n t) -> p n t", n=NL)
            q0m = pool.tile([32, NL], dt)
            for t in range(T):
                vn = q0s[:, :, t]
                nc.gpsimd.tensor_tensor(vn, q0[:], tq0v[:, :, t], op.add)
                nc.gpsimd.tensor_scalar(q0m[:], vn, 1.0, None, op.is_lt)
                nc.gpsimd.tensor_tensor(q0[:], q0m[:], vn, op.mult)
            s5 = q0s[:].rearrange("p n t -> p (n t)")
            nc.gpsimd.tensor_scalar(s5, s5, 1.0, None, op.is_ge)

            # g1 filter + q1 inputs (PE 32x32) + sb1
            nc.vector.tensor_tensor_scan(g1s[:], mask1[0:32, :], s5, 0.0,
                                         op.mult, op.add)
            nc.tensor.matmul(q1_ps[:], spk[:, 3:35], g1s[:])
            nc.vector.tensor_scalar(tq1[:], q1_ps[:], spk[:, 2:3], None,
                                    op.add)

            # ---------------- phase 7: q1 IF chain -----------------------
            tq1v = tq1[:].rearrange("p (n t) -> p n t", n=NL)
            for t in range(T):
                vn = q1s[:, :, t]
                nc.vector.tensor_tensor(vn, q1[:], tq1v[:, :, t], op.add)
                nc.vector.scalar_tensor_tensor(q1[:], vn, 1.0, vn,
                                               op.is_lt, op.mult)
            s6 = q1s[:].rearrange("p n t -> p (n t)")
            nc.gpsimd.tensor_scalar(s6, s6, 1.0, None, op.is_ge)

            # g2 filter + q2 contributions (PE 32x1) + sb2, then cumsum
            nc.vector.tensor_tensor_scan(g2s[:], mask1[0:32, :], s6, 0.0,
                                         op.mult, op.add)
            nc.tensor.matmul(q2_ps[:], spk[:, 0:1], g2s[:])
            nc.vector.tensor_scalar(tq2[:], q2_ps[:], srow[:, 32:33], None,
                                    op.add)
            nc.vector.tensor_tensor_scan(q2s[:], cmask[:], tq2[:], 0.0,
                                         op.mult, op.add)

            # output: q2s[0, n*T+t] -> out[n, t] (contiguous)
            nc.sync.dma_start(
                out_d[:].rearrange("n t -> (n t)").unsqueeze(0), q2s[:])

    nc.compile()
    return nc, out_d.name


def _prep_core_inputs(x, w1, b1, w2, b2, sw0, sb0, sw1, sb1, sw2, sb2,
                      core):
    """Host-side marshalling of one core's shard into device layouts."""
    n0 = core * NL
    inv = F32(1.0) / TAU
    xs = x[:, n0:n0 + NL]                      # (T, NL, 2, C)
    x0 = np.ascontiguousarray(np.moveaxis(xs[:, :, 0, :], [0, 1, 2],
                                          [2, 1, 0]))   # (C, NL, T)
    x1 = np.ascontiguousarray(np.moveaxis(xs[:, :, 1, :], [0, 1, 2],
                                          [2, 1, 0]))
    x0R = np.zeros((C, NL, 128), F32)
    x0R[:, :, :T] = x0[:, :, ::-1] * inv       # x0R[...,tau']=x0[63-tau']/tau
    x1P = np.zeros((C, NL, 128), F32)
    x1P[:, :, 63:127] = x1 * inv               # x1P[...,tau]=x1[tau-63]/tau

    dist = np.arange(D) - D // 2
    kint = (1.0 / (1.0 - np.exp(-np.abs(dist) / 2.0)))
    kint[D // 2] = 1.0 / (1.0 - np.exp(-1.0 / 2.0))
    kint = np.broadcast_to(kint.astype(F32), (C, D)).copy()

    wpk = np.zeros((C, 32), F32)
    wpk[:, 0:10] = w1[:, 0]
    wpk[:, 10:20] = b1
    wpk[:, 20:30] = w2[0, :]
    wpk[:, 30] = b2[0]
    wpk[:, 31] = 1.0

    spk = np.zeros((32, 35), F32)
    spk[:, 0] = sw2[0, :]
    spk[:, 1] = sb0
    spk[:, 2] = sb1
    spk[:, 3:35] = sw1.T
    srow = np.zeros((1, 33), F32)
    srow[0, 0:32] = sw0[:, 0]
    srow[0, 32] = sb2[0]

    return {
        "x0r": x0R, "x1p": x1P, "kint": kint, "wpk": wpk, "spk": spk,
        "srow": srow,
    }


def kernel(x, w1, b1, w2, b2, sw0, sb0, sw1, sb1, sw2, sb2):
    from concourse.bass_utils import run_bass_kernel_spmd

    if "prog" not in _cache:
        _cache["prog"] = _build_program()
    nc, out_name = _cache["prog"]

    args = (x.astype(F32), w1, b1, w2, b2, sw0, sb0, sw1, sb1, sw2, sb2)
    in_maps = [_prep_core_inputs(*args, core) for core in range(NCORES)]
    res = run_bass_kernel_spmd(nc, in_maps, core_ids=list(range(NCORES)))
    out = np.concatenate([r[out_name] for r in res.results], axis=0)
    # device layout is (N, T); reference returns (T, N, 1)
    return np.ascontiguousarray(out.T)[:, :, None].astype(F32)


if __name__ == "__main__":
    d = np.load("/tmp/inputs.npz")
    out = kernel(**{k: d[k] for k in d.files})
    print("kernel out", out.shape, float(np.abs(out).max()))
